# revision 1
# baseline (speedup 1.0000x reference)
"""Masked multi-head self-attention on 8 Trainium2 NeuronCores.

Problem: B=4, T=1024, C=1024, H=16 heads (D=64), key-padding mask.
Sharding: core c handles batch b=c//2 and heads [8*(c%2), 8*(c%2)+8)
(data parallel on B x tensor parallel on heads). Each core computes its
partial output projection; host sums the two head-half partials per batch
and adds bp.

Per-core device algorithm (everything in "transposed" layouts so the
contraction dim always sits on SBUF partitions):
  QT = Wq_c^T x_b^T   [512, T]   (head dim on partitions)
  KT = Wk_c^T x_b^T   [512, T]
  V  = x_b Wv_c       [T, 512]   (T on partitions), augmented per head with
                                 ones columns so att@v also yields softmax sums
  S^T_h = KT_h^T QT_h scaled 1/8, key-pad mask applied as per-partition
          bias (-1e9) inside the ScalarE exp -> expS (bf16)
  y_aug^T_h = V_aug_h^T expS_h   (PSUM accum over key tiles)
  normalize with reciprocal sums (broadcast across partitions via a
  selector matmul), then out_partial = y^T^T Wp_c.

Fully-padded query rows (reference softmaxes an all -1e9 row => uniform
attention over ALL keys) are fixed up on the host:
  out[b, q_pad, :] = (mean_k x[b]) @ Wv @ Wp + bv @ Wp + bp.
"""

import sys

sys.path.insert(0, "/opt/trn_rl_repo")

import math

import ml_dtypes
import numpy as np

import concourse.bass as bass
import concourse.tile as tile
from concourse import mybir
from concourse.bass_utils import run_bass_kernel_spmd

B, T, C, H = 4, 1024, 1024, 16
D = C // H          # 64 head dim
HL = H // 2         # 8 heads per core
CP = HL * D         # 512 per-core projection width
P = 128
KT = C // P         # 8 contraction subtiles
MT = CP // P        # 4 m-tiles of QT/KT
TTL = T // P        # 8 T tiles
NCH = T // 512      # 2 free-dim chunks of T
BF16 = mybir.dt.bfloat16
F32 = mybir.dt.float32

LAST_RESULTS = None  # BassKernelResults of the most recent run (for test.py)


# ---------------------------------------------------------------------------
# Workaround: this walrus build only accepts ONE sync-wait command per
# instruction, but Tile's sem assignment can attach several. Post-pass: move
# extra waits onto fresh same-engine nops inserted just before the carrier.
def _split_multi_waits(nc):
    n = 0
    for f in nc.m.functions:
        for blk in f.blocks:
            newlist, changed = [], False
            for i in blk.instructions:
                si = i.sync_info
                if si is not None and si.on_wait is not None and len(si.on_wait) > 1:
                    w = list(si.on_wait)
                    for ww in w[:-1]:
                        newlist.append(
                            mybir.InstNoOp(
                                name=f"WSPLIT-{n}",
                                engine=i.engine,
                                sync_info=mybir.SyncInfo(on_wait=[ww], on_update=[]),
                            )
                        )
                        n += 1
                    si.on_wait = [w[-1]]
                    changed = True
                newlist.append(i)
            if changed:
                blk.instructions = newlist


# NTFF profiling hook: bass_utils' axon trace path looks for
# antenv.axon_hooks, which this image lacks. Synthesize it and register the
# ctypes-based profiler from trn_agent_boot so BASS_TRACE=1 yields exec times.
def _register_ntff_hook():
    try:
        import antenv.axon_hooks  # noqa: F401
        return
    except ImportError:
        pass
    try:
        import types

        import antenv
        from trn_agent_boot.trn_boot import _ntff_profile_via_ctypes

        mod = types.ModuleType("antenv.axon_hooks")
        _state = {"hook": None}
        mod.set_axon_ntff_profile_hook = lambda h: _state.__setitem__("hook", h)
        mod.get_axon_ntff_profile_hook = lambda: _state["hook"]
        sys.modules["antenv.axon_hooks"] = mod
        antenv.axon_hooks = mod
        so = "/opt/axon/libaxon_pjrt.so"
        import os

        if os.path.exists(so):
            mod.set_axon_ntff_profile_hook(_ntff_profile_via_ctypes(so))
    except Exception:
        pass


_register_ntff_hook()
# ---------------------------------------------------------------------------


def _build_nc():
    nc = bass.Bass()
    xT = nc.dram_tensor("xT", [C, T], BF16, kind="ExternalInput")
    wq = nc.dram_tensor("wq", [C, CP], BF16, kind="ExternalInput")
    wk = nc.dram_tensor("wk", [C, CP], BF16, kind="ExternalInput")
    wv = nc.dram_tensor("wv", [C, CP], BF16, kind="ExternalInput")
    wp = nc.dram_tensor("wp", [CP, C], BF16, kind="ExternalInput")
    bq = nc.dram_tensor("bq", [P, MT], F32, kind="ExternalInput")
    bk = nc.dram_tensor("bk", [P, MT], F32, kind="ExternalInput")
    bv = nc.dram_tensor("bv", [P, CP], F32, kind="ExternalInput")
    mk = nc.dram_tensor("mk", [P, KT], F32, kind="ExternalInput")
    ef = nc.dram_tensor("ef", [HL, CP], BF16, kind="ExternalInput")
    out = nc.dram_tensor("out", [T, C], F32, kind="ExternalOutput")

    EXP = mybir.ActivationFunctionType.Exp

    with tile.TileContext(nc) as tc:
        with (
            tc.tile_pool(name="consts", bufs=1) as consts,
            tc.tile_pool(name="expp", bufs=20) as expp,
            tc.tile_pool(name="outp", bufs=3) as outp,
            tc.tile_pool(name="ps2", bufs=2, space="PSUM") as ps2,
            tc.tile_pool(name="psy", bufs=2, space="PSUM") as psy,
            tc.tile_pool(name="ps1", bufs=2, space="PSUM") as ps1,
        ):
            # ---- input DMAs (chunked + spread across engine queues) ---------
            xTr = xT.rearrange("(kt p) t -> p kt t", p=P)
            xT_sb = consts.tile([P, KT, T], BF16)
            for kt in range(KT):
                nc.sync.dma_start(xT_sb[:, kt, :], xTr[:, kt, :])
            wq_sb = consts.tile([P, KT, CP], BF16)
            nc.scalar.dma_start(wq_sb[:], wq.rearrange("(kt p) n -> p kt n", p=P))
            wk_sb = consts.tile([P, KT, CP], BF16)
            nc.scalar.dma_start(wk_sb[:], wk.rearrange("(kt p) n -> p kt n", p=P))
            wv_sb = consts.tile([P, KT, CP], BF16)
            nc.sync.dma_start(wv_sb[:], wv.rearrange("(kt p) n -> p kt n", p=P))
            bq_sb = consts.tile([P, MT], F32)
            nc.scalar.dma_start(bq_sb[:], bq[:])
            bk_sb = consts.tile([P, MT], F32)
            nc.scalar.dma_start(bk_sb[:], bk[:])
            bv_sb = consts.tile([P, CP], F32)
            nc.sync.dma_start(bv_sb[:], bv[:])
            mk_sb = consts.tile([P, KT], F32)
            nc.scalar.dma_start(mk_sb[:], mk[:])
            wp_sb = consts.tile([P, MT, T], BF16)
            nc.gpsimd.dma_start(wp_sb[:], wp.rearrange("(s p) n -> p s n", p=P))
            ef_sb = consts.tile([HL, CP], BF16)
            nc.gpsimd.dma_start(ef_sb[:], ef[:])

            # ---- persistent SBUF tensors ------------------------------------
            # V_aug layout [p, kt, h, m]: even h -> v at m 0:64, ones col at 96;
            # odd h -> ones col at 32, v at m 64:128; rest zero.
            V_sb = consts.tile([P, KT, HL, P], BF16)
            QT_sb = consts.tile([P, MT, T], BF16)
            KT_sb = consts.tile([P, MT, T], BF16)
            y_sb = consts.tile([P, MT, T], BF16)
            # per-head softmax sums staged at (lane 96, block h//2) for even
            # heads and (lane 32, block h//2) for odd heads
            stage = consts.tile([P, MT, T], BF16)
            sums8 = consts.tile([HL, T], BF16)
            lns8 = consts.tile([HL, T], F32)
            rcp8 = consts.tile([HL, T], BF16)

            V5 = V_sb.rearrange("p kt (hh par) m -> p kt hh par m", par=2)
            nc.gpsimd.memset(V_sb[:, 0:4], 0.0)
            nc.vector.memset(V_sb[:, 4:8], 0.0)
            nc.gpsimd.memset(V5[:, :, :, 0, 96:97], 1.0)
            nc.gpsimd.memset(V5[:, :, :, 1, 32:33], 1.0)

            def qk_proj(mt):
                for w_sb, b_sb, dst in ((wq_sb, bq_sb, QT_sb), (wk_sb, bk_sb, KT_sb)):
                    pss = [ps1.tile([P, 512], F32, tag="ps1", name=f"qkps{ch}") for ch in range(NCH)]
                    for kt in range(KT):
                        for ch in range(NCH):
                            nc.tensor.matmul(
                                pss[ch][:],
                                w_sb[:, kt, mt * P : (mt + 1) * P],
                                xT_sb[:, kt, ch * 512 : (ch + 1) * 512],
                                start=(kt == 0),
                                stop=(kt == KT - 1),
                            )
                    for ch in range(NCH):
                        nc.vector.tensor_scalar_add(
                            dst[:, mt, ch * 512 : (ch + 1) * 512],
                            pss[ch][:], b_sb[:, mt : mt + 1],
                        )

            qk_proj(0)

            # ---- V projection (natural layout, T on partitions) -------------
            for tt in range(TTL):
                psv = ps1.tile([P, 512], F32, tag="ps1")
                for kt in range(KT):
                    nc.tensor.matmul(
                        psv[:],
                        xT_sb[:, kt, tt * P : (tt + 1) * P],
                        wv_sb[:, kt, :],
                        start=(kt == 0),
                        stop=(kt == KT - 1),
                    )
                src = psv.rearrange("p (hh par d) -> p par hh d", par=2, d=D)
                bvr = bv_sb.rearrange("p (hh par d) -> p par hh d", par=2, d=D)
                nc.vector.tensor_add(V5[:, tt, :, 0, 0:D], src[:, 0], bvr[:, 0])
                nc.vector.tensor_add(V5[:, tt, :, 1, D:P], src[:, 1], bvr[:, 1])

            # ---- attention per head pair ------------------------------------
            for p in range(MT):
                hA, hB = 2 * p, 2 * p + 1
                eAs, eBs = [], []
                for kt in range(KT):
                    psA = ps2.tile([P, T], F32, tag="ps2")
                    psB = ps2.tile([P, T], F32, tag="ps2")
                    for ch in range(NCH):
                        sl = slice(ch * 512, (ch + 1) * 512)
                        nc.tensor.matmul(
                            psA[:, sl],
                            KT_sb[0:D, p, kt * P : (kt + 1) * P],
                            QT_sb[0:D, p, sl],
                            start=True,
                            stop=True,
                        )
                        nc.tensor.matmul(
                            psB[:, sl],
                            KT_sb[D:P, p, kt * P : (kt + 1) * P],
                            QT_sb[D:P, p, sl],
                            start=True,
                            stop=True,
                        )
                    eA = expp.tile([P, T], BF16, tag="exp")
                    nc.scalar.activation(
                        eA[:], psA[:], EXP,
                        bias=mk_sb[:, kt : kt + 1], scale=1.0 / math.sqrt(D),
                    )
                    eB = expp.tile([P, T], BF16, tag="exp")
                    nc.scalar.activation(
                        eB[:], psB[:], EXP,
                        bias=mk_sb[:, kt : kt + 1], scale=1.0 / math.sqrt(D),
                    )
                    eAs.append(eA)
                    eBs.append(eB)
                for h, es in ((hA, eAs), (hB, eBs)):
                    yps = [psy.tile([P, 512], F32, tag="psy", name=f"yps{ch}") for ch in range(NCH)]
                    for kt in range(KT):
                        for ch in range(NCH):
                            nc.tensor.matmul(
                                yps[ch][:], V_sb[:, kt, h, :],
                                es[kt][:, ch * 512 : (ch + 1) * 512],
                                start=(kt == 0), stop=(kt == KT - 1),
                            )
                    # harvest: y rows + ones-row softmax sums (lane-aligned)
                    lane = 96 if h % 2 == 0 else 32
                    yr = slice(0, D) if h % 2 == 0 else slice(D, P)
                    for ch in range(NCH):
                        sl = slice(ch * 512, (ch + 1) * 512)
                        nc.vector.tensor_copy(y_sb[yr, p, sl], yps[ch][yr, :])
                        nc.vector.tensor_copy(
                            stage[lane : lane + 1, p, sl], yps[ch][lane : lane + 1, :]
                        )
                        nc.sync.dma_start(
                            sums8[h : h + 1, sl], stage[lane : lane + 1, p, sl]
                        )
                if p + 1 < MT:
                    qk_proj(p + 1)

            # ---- reciprocal of sums via exp(-ln(s)), broadcast, normalize ---
            nc.scalar.activation(
                lns8[0:HL, :], sums8[0:HL, :], mybir.ActivationFunctionType.Ln
            )
            nc.scalar.activation(rcp8[0:HL, :], lns8[0:HL, :], EXP, scale=-1.0)
            for p in range(MT):
                bps = ps2.tile([P, T], F32, tag="ps2")
                for ch in range(NCH):
                    sl = slice(ch * 512, (ch + 1) * 512)
                    nc.tensor.matmul(
                        bps[:, sl], ef_sb[0:HL, p * P : (p + 1) * P],
                        rcp8[0:HL, sl], start=True, stop=True,
                    )
                nc.vector.tensor_mul(y_sb[:, p, :], y_sb[:, p, :], bps[:])


            # ---- output projection ------------------------------------------
            for tt in range(TTL):
                pps = [ps1.tile([P, 512], F32, tag="ps1", name=f"pjps{ch}") for ch in range(NCH)]
                for s2 in range(MT):
                    for ch in range(NCH):
                        nc.tensor.matmul(
                            pps[ch][:],
                            y_sb[:, s2, tt * P : (tt + 1) * P],
                            wp_sb[:, s2, ch * 512 : (ch + 1) * 512],
                            start=(s2 == 0), stop=(s2 == MT - 1),
                        )
                for ch in range(NCH):
                    sl = slice(ch * 512, (ch + 1) * 512)
                    ot = outp.tile([P, 512], F32, tag="out")
                    nc.vector.tensor_copy(ot[:], pps[ch][:])
                    nc.sync.dma_start(out[tt * P : (tt + 1) * P, sl], ot[:])
    _split_multi_waits(nc)
    return nc


_NC = None


def _get_nc():
    global _NC
    if _NC is None:
        _NC = _build_nc()
    return _NC


def kernel(x, x_padding_judge, Wq, bq, Wk, bk, Wv, bv, Wp, bp):
    global LAST_RESULTS
    x = np.asarray(x, dtype=np.float32)
    pad = np.asarray(x_padding_judge, dtype=np.float32)
    Wq = np.asarray(Wq, dtype=np.float32)
    Wk = np.asarray(Wk, dtype=np.float32)
    Wv = np.asarray(Wv, dtype=np.float32)
    Wp = np.asarray(Wp, dtype=np.float32)
    bq = np.asarray(bq, dtype=np.float32)
    bk = np.asarray(bk, dtype=np.float32)
    bv = np.asarray(bv, dtype=np.float32)
    bp = np.asarray(bp, dtype=np.float32)
    bf = ml_dtypes.bfloat16

    # selector matrix for broadcasting per-head softmax sums: within each
    # 128-wide m-tile, partitions 0:64 take the even head's sums (staged at
    # lane 96), partitions 64:128 the odd head's (staged at lane 32)
    efm = np.zeros((HL, CP), dtype=np.float32)
    for m in range(CP):
        efm[2 * (m // P) + (m % P) // D, m] = 1.0

    in_maps = []
    for c in range(8):
        b, s = c // 2, c % 2
        cols = slice(s * CP, (s + 1) * CP)
        in_maps.append({
            "xT": np.ascontiguousarray(x[b].T).astype(bf),
            "wq": Wq[:, cols].astype(bf),
            "wk": Wk[:, cols].astype(bf),
            "wv": Wv[:, cols].astype(bf),
            "wp": Wp[cols, :].astype(bf),
            "bq": np.ascontiguousarray(bq[cols].reshape(MT, P).T),
            "bk": np.ascontiguousarray(bk[cols].reshape(MT, P).T),
            "bv": np.broadcast_to(bv[cols], (P, CP)).copy(),
            "mk": np.ascontiguousarray((-1e9 * pad[b]).reshape(KT, P).T),
            "ef": efm.astype(bf),
        })

    res = run_bass_kernel_spmd(_get_nc(), in_maps, core_ids=list(range(8)))
    LAST_RESULTS = res

    out = np.empty((B, T, C), dtype=np.float32)
    for b in range(B):
        out[b] = res.results[2 * b]["out"] + res.results[2 * b + 1]["out"] + bp

    # fully-padded query rows: uniform attention over ALL keys
    for b in range(B):
        rows = np.nonzero(pad[b] == 1.0)[0]
        if rows.size:
            xbar = x[b].mean(axis=0)
            out[b, rows, :] = (xbar @ Wv + bv) @ Wp + bp
    return out

